# revision 70
# baseline (speedup 1.0000x reference)
"""Trainium2 Bass kernel for single-head causal attention.

x:[4,4096,1024] f32, W_q/W_k/W_v:[1024,64], W_o:[64,1024].

Sharding: 8 cores = 4 batches x 2 roles. The 4096 queries of a batch are
split into 16 spans of 256; role A takes spans [0,3,4,7,8,11,12,15], role B
takes [1,2,5,6,9,10,13,14] (zig-zag), so both roles see an identical causal
workload profile: slot i (i=0..7, spans sorted by span index) attends to
exactly B_i = 4*(i+1) key chunks of 128. Per-core differences (which spans,
the causal-boundary masks) are carried purely in input data; the compiled
program is one SPMD binary.

All matmuls run in bf16 (1 cycle/row on the PE vs 4 for fp32) with fp32 PSUM
accumulation. Scores for 4 key chunks land in one [128,1024] PSUM quad and
get a single Exp activation; only the last quad of each slot needs a mask
multiply (covers both the causal diagonal and the dead blocks of the role
with the smaller span). Projection and output work for neighbouring slots is
interleaved between attention quads so the PE stays dense while the
activation engine paces the exp stream.
"""

import sys

for _p in ("/opt/trn_rl_repo",):
    if _p not in sys.path:
        sys.path.insert(0, _p)

import numpy as np

D_MODEL = 1024
D_HEAD = 64
SEQ = 4096
BATCH = 4
NCORES = 8
NQ = 2048           # queries per core
P = 128
DCH = D_MODEL // P  # 8 contraction chunks
SPAN = 256          # queries per slot
NSLOT = NQ // SPAN  # 8 slots per core
ROLE_A = [0, 3, 4, 7, 8, 11, 12, 15]
ROLE_B = [1, 2, 5, 6, 9, 10, 13, 14]

_prog = None


def _build_program():
    import concourse.bacc as bacc
    import concourse.mybir as mybir
    import concourse.tile as tile
    from concourse.masks import make_identity

    fp32 = mybir.dt.float32
    bf16 = mybir.dt.bfloat16
    nc = bacc.Bacc("TRN2", target_bir_lowering=False, debug=False)

    xt = nc.dram_tensor("xt", [D_MODEL, SEQ], bf16, kind="ExternalInput")
    xtq = nc.dram_tensor("xtq", [D_MODEL, NQ], bf16, kind="ExternalInput")
    wqkv = nc.dram_tensor("wqkv", [D_MODEL, 2 * P], bf16, kind="ExternalInput")
    wo = nc.dram_tensor("wo", [D_HEAD, D_MODEL], bf16, kind="ExternalInput")
    msk = nc.dram_tensor("msk", [P, NSLOT, 1024], bf16, kind="ExternalInput")
    y = nc.dram_tensor("y", [NQ, D_MODEL], bf16, kind="ExternalOutput")

    Exp = mybir.ActivationFunctionType.Exp

    with tile.TileContext(nc) as tc:
        with (
            tc.tile_pool(name="singles", bufs=1) as singles,
            tc.tile_pool(name="work", bufs=5) as work,
            tc.tile_pool(name="pwork", bufs=4) as pwork,
            tc.tile_pool(name="ywork", bufs=3) as ywork,
            tc.tile_pool(name="mm_ps", bufs=2, space="PSUM") as mm_ps,
            tc.tile_pool(name="s_ps", bufs=2, space="PSUM") as s_ps_pool,
            tc.tile_pool(name="pv_ps", bufs=2, space="PSUM") as pv_pool,
        ):
            # ---- persistent SBUF tensors ----
            kvt = singles.tile([P, SEQ], bf16, tag="kvt")  # 0:64 K^T, 64:128 V^T
            vaug = singles.tile([P, SEQ // P, D_HEAD + 1], bf16, tag="vaug")
            qt_sb = singles.tile([D_HEAD, NQ], bf16, tag="qt_sb")  # Q^T pre-scaled
            otb = singles.tile([D_HEAD, NQ], bf16, tag="otb")      # O^T (unscaled)
            rden = singles.tile([1, NQ], fp32, tag="rden")         # 1/den per query
            rdent = singles.tile([P, NQ // P], fp32, tag="rdent")  # transposed
            wpack_sb = singles.tile([P, DCH, 2 * P], bf16, tag="wpack_sb")
            wkv_sb = wpack_sb[:, :, 0:P]
            wq_sb = wpack_sb[:, :, P : P + D_HEAD]
            wo_sb = singles.tile([D_HEAD, D_MODEL], bf16, tag="wo_sb")
            msk_sb = singles.tile([P, NSLOT, 1024], bf16, tag="msk_sb")
            ident = singles.tile([P, D_HEAD], bf16, tag="ident")
            one_sb = singles.tile([1, 1], fp32, tag="one_sb")

            xc_t = [None] * 8   # x chunk tiles (keys)
            xq_t = [None] * 4   # x chunk tiles (queries)
            pv_t = [None] * 8   # per-slot PV accumulators
            kv_ps = {}
            q_ps = {}

            def dma_xc(c, first):
                """Load x^T key chunk c. The first chunk is split so the dc=0
                slab (all the first matmul needs) lands early."""
                src = xt[:, c * 512 : (c + 1) * 512].rearrange(
                    "(c p) m -> p c m", p=P
                )
                t = work.tile([P, DCH, 512], bf16, tag="xchunk", name=f"xc{c}")
                if first:
                    nc.sync.dma_start(out=t[:, 0:1, :], in_=src[:, 0:1, :])
                    nc.sync.dma_start(out=t[:, 1:4, :], in_=src[:, 1:4, :])
                    nc.sync.dma_start(out=t[:, 4:DCH, :], in_=src[:, 4:DCH, :])
                else:
                    nc.sync.dma_start(out=t[:, 0:4, :], in_=src[:, 0:4, :])
                    nc.sync.dma_start(out=t[:, 4:DCH, :], in_=src[:, 4:DCH, :])
                xc_t[c] = t

            def dma_xq(c, split=False):
                src_ = xtq[:, c * 512 : (c + 1) * 512].rearrange(
                    "(c p) m -> p c m", p=P
                )
                t = work.tile([P, DCH, 512], bf16, tag="xchunk", name=f"xq{c}")
                # column halves: each slot's queries land separately, so
                # the first half's projection never waits for the second
                nc.sync.dma_start(out=t[:, :, 0:256], in_=src_[:, :, 0:256])
                nc.sync.dma_start(out=t[:, :, 256:512], in_=src_[:, :, 256:512])
                xq_t[c] = t

            def dma_msk(s):
                nc.sync.dma_start(
                    out=msk_sb[:, s : s + 1, :], in_=msk[:, s : s + 1, :]
                )

            def proj_units(c):
                """K/V (+Q) projection for x chunk c, as small closures."""
                units = []

                def kv_mm(dc):
                    def go():
                        if dc == 0:
                            kv_ps[c] = mm_ps.tile([P, 512], fp32, tag="mm", name=f"kvps{c}")
                        nc.tensor.matmul(
                            kv_ps[c],
                            lhsT=wkv_sb[:, dc, :],
                            rhs=xc_t[c][:, dc, :],
                            start=(dc == 0),
                            stop=(dc == DCH - 1),
                            skip_group_check=True,
                        )
                    return go

                units += [kv_mm(dc) for dc in range(DCH)]
                units.append(
                    lambda: nc.vector.tensor_copy(
                        out=kvt[:, c * 512 : (c + 1) * 512], in_=kv_ps[c]
                    )
                )

                def transp():
                    tp = mm_ps.tile([P, 4, D_HEAD], bf16, tag="mm", name=f"tp{c}")
                    for t in range(4):
                        kc = c * 4 + t
                        nc.tensor.transpose(
                            tp[:, t, :],
                            kvt[D_HEAD:P, kc * P : (kc + 1) * P],
                            ident[D_HEAD:P, :],
                        )
                    nc.vector.tensor_copy(
                        out=vaug[:, c * 4 : c * 4 + 4, 0:D_HEAD], in_=tp
                    )

                units.append(transp)

                return units

            def qproj_units(c):
                """Q^T projection for query chunk c (slots 2c, 2c+1)."""
                units = []
                halves = ((0, 256), (256, 512))

                def q_mm(dc, h0, h1):
                    def go():
                        if dc == 0 and h0 == 0:
                            q_ps[c] = mm_ps.tile([D_HEAD, 512], fp32, tag="mm", name=f"qps{c}")
                        nc.tensor.matmul(
                            q_ps[c][:, h0:h1],
                            lhsT=wq_sb[:, dc, :],
                            rhs=xq_t[c][:, dc, h0:h1],
                            start=(dc == 0),
                            stop=(dc == DCH - 1),
                            skip_group_check=True,
                        )
                    return go

                def q_cp(h0, h1):
                    def go():
                        nc.vector.tensor_copy(
                            out=qt_sb[:, c * 512 + h0 : c * 512 + h1],
                            in_=q_ps[c][:, h0:h1],
                        )
                    return go

                for h0, h1 in halves:
                    units += [q_mm(dc, h0, h1) for dc in range(DCH)]
                    units.append(q_cp(h0, h1))
                return units

            def pre_units(s):
                """Per-slot epilogue part 1: free the PV accumulator —
                reciprocal of the denominator row, its transpose, O^T cast."""
                pv = pv_t[s]
                pr0 = s * SPAN
                units = [
                    lambda: nc.vector.reciprocal(
                        rden[:, pr0 : pr0 + SPAN], pv[D_HEAD : D_HEAD + 1, :]
                    ),
                ]

                def rdt():
                    rt_ps = mm_ps.tile([P, 2], fp32, tag="mm", name=f"rt{s}")
                    for k in range(2):
                        qti = s * 2 + k
                        nc.tensor.matmul(
                            rt_ps[:, k : k + 1],
                            lhsT=rden[:, qti * P : (qti + 1) * P],
                            rhs=one_sb,
                            start=True,
                            stop=True,
                        )
                    nc.vector.tensor_copy(
                        out=rdent[:, s * 2 : s * 2 + 2], in_=rt_ps
                    )

                units.append(rdt)
                units.append(
                    lambda: nc.vector.tensor_copy(
                        out=otb[:, pr0 : pr0 + SPAN], in_=pv[0:D_HEAD, :]
                    )
                )
                return units

            def out_units(s):
                """Per-slot epilogue part 2 (schedulable anywhere after
                pre_units(s)): out-projection, 1/den scaling, store. The
                final slot is latency-critical (nothing left to overlap
                with), so it takes wider PSUM tiles from the by-then-idle
                score pool, splits the scaling across DVE and ACT, and
                stores each half-row as soon as it is scaled."""
                last = s == 7
                yb_t = {}
                yp_t = {}

                def oproj(k, no):
                    def go():
                        qti = s * 2 + k
                        if no == 0:
                            yb_t[k] = ywork.tile([P, D_MODEL], bf16, tag="ysb", name=f"yb{s}_{k}")
                            if last:
                                yp_t[k] = s_ps_pool.tile(
                                    [P, 1024], fp32, tag="s", name=f"ypl{k}"
                                )
                        if last:
                            yp = yp_t[k][:, no * 512 : (no + 1) * 512]
                        else:
                            yp = mm_ps.tile([P, 512], fp32, tag="mm", name=f"yp{s}_{k}_{no}")
                        nc.tensor.matmul(
                            yp,
                            lhsT=otb[:, qti * P : (qti + 1) * P],
                            rhs=wo_sb[:, no * 512 : (no + 1) * 512],
                            start=True,
                            stop=True,
                        )
                        dst = yb_t[k][:, no * 512 : (no + 1) * 512]
                        if last and no == 1:
                            nc.scalar.activation(
                                dst,
                                yp,
                                mybir.ActivationFunctionType.Copy,
                                scale=rdent[:, qti : qti + 1],
                            )
                        else:
                            nc.vector.tensor_scalar_mul(
                                dst, yp, rdent[:, qti : qti + 1]
                            )
                        if last:
                            nc.sync.dma_start(
                                out=y[
                                    qti * P : (qti + 1) * P,
                                    no * 512 : (no + 1) * 512,
                                ],
                                in_=dst,
                            )
                        elif no == 1:
                            nc.sync.dma_start(
                                out=y[qti * P : (qti + 1) * P, :], in_=yb_t[k]
                            )
                    return go

                return [oproj(0, 0), oproj(0, 1), oproj(1, 0), oproj(1, 1)]

            # ---- prologue ----
            make_identity(nc, ident[D_HEAD:P, :])
            nc.vector.memset(one_sb, 1.0)
            nc.vector.memset(vaug[:, :, D_HEAD : D_HEAD + 1], 1.0)
            w_src = wqkv.rearrange("(c p) m -> p c m", p=P)
            nc.sync.dma_start(out=wpack_sb, in_=w_src)
            dma_xc(0, first=True)
            dma_xq(0, split=True)
            for u in proj_units(0) + qproj_units(0):
                u()

            # ---- main loop: attention(slot sc) + interleaved fillers ----
            # iteration sc hosts: projection of chunk sc+1, slot sc-1's PV
            # epilogue, and slot sc-2's output (deferred so the late,
            # exp-bound iterations get PE filler work); chunk DMAs are
            # issued two iterations ahead of their projection.
            for sc in range(8):
                if sc == 0:
                    dma_xc(1, first=False)
                    dma_msk(0)
                    dma_xc(2, first=False)
                    dma_xq(1)
                    dma_xq(2)
                    dma_msk(1)
                elif sc == 1:
                    dma_xc(3, first=False)
                    dma_xq(3)
                    nc.sync.dma_start(out=wo_sb, in_=wo[:, :])
                elif sc < 6:
                    dma_xc(sc + 2, first=False)

                # ready-first: pre/out inputs completed last iteration,
                # proj inputs (the prefetched chunk) arrive mid-iteration
                units = []
                if sc >= 1:
                    units += pre_units(sc - 1)
                if sc >= 2:
                    units += out_units(sc - 2)
                if sc < 7:
                    units += proj_units(sc + 1)
                    if sc + 1 < 4:
                        units += qproj_units(sc + 1)
                tail_units = out_units(6) if sc == 7 else []

                B = 4 * (sc + 1)
                pv_t[sc] = pv_pool.tile(
                    [D_HEAD + 1, SPAN], fp32, tag="pv", name=f"pv{sc}"
                )
                pvh = pv_t[sc]
                qs_ap = qt_sb[:, sc * SPAN : (sc + 1) * SPAN]

                n_qd = sc + 1
                sq_t = [None] * n_qd
                p_t = [None] * n_qd

                def issue_qk(qd):
                    sq = s_ps_pool.tile([P, 1024], fp32, tag="s", name=f"sq{sc}_{qd}")
                    for t in range(4):
                        kc = 4 * qd + t
                        nc.tensor.matmul(
                            sq[:, t * SPAN : (t + 1) * SPAN],
                            lhsT=kvt[0:D_HEAD, kc * P : (kc + 1) * P],
                            rhs=qs_ap,
                            start=True,
                            stop=True,
                        )
                    sq_t[qd] = sq

                def issue_exp(qd):
                    p = pwork.tile([P, 1024], bf16, tag="p", name=f"p{sc}_{qd}")
                    nc.scalar.activation(p, sq_t[qd], Exp)
                    if qd == sc:  # last quad: causal boundary + dead blocks
                        nc.vector.tensor_tensor(
                            p, p, msk_sb[:, sc % 2, :], mybir.AluOpType.mult
                        )
                    p_t[qd] = p

                # masked quad second-to-last: off the slot-end critical
                # chain, but late enough that chunk sc's projection (a filler
                # in the previous iteration) has certainly landed.
                order = list(range(sc)) + [sc]
                if sc >= 1:
                    order[sc - 1], order[sc] = order[sc], order[sc - 1]

                def issue_pv(pos):
                    qd = order[pos]
                    for t in range(4):
                        kc = 4 * qd + t
                        nc.tensor.matmul(
                            pvh,
                            lhsT=vaug[:, kc, :],
                            rhs=p_t[qd][:, t * SPAN : (t + 1) * SPAN],
                            start=(pos == 0 and t == 0),
                            stop=(pos == n_qd - 1 and t == 3),
                            skip_group_check=True,
                        )

                # spread filler units across the quad stream
                ui = 0

                def fill(frac):
                    nonlocal ui
                    tgt = int(round(frac * len(units)))
                    while ui < tgt:
                        units[ui]()
                        ui += 1

                issue_qk(order[0])
                issue_exp(order[0])
                for pos in range(1, n_qd):
                    fill(pos / n_qd)
                    issue_qk(order[pos])
                    issue_pv(pos - 1)
                    issue_exp(order[pos])
                issue_pv(n_qd - 1)
                fill(1.0)
                for u in tail_units:
                    u()

            for u in pre_units(7) + out_units(7):
                u()

    nc.finalize()
    return nc


def _get_program():
    global _prog
    if _prog is None:
        _prog = _build_program()
    return _prog


def _make_mask(role_spans):
    """[128, 8, 1024] f32: slot i's last-quad window (key chunks 4i..4i+3)."""
    out = np.zeros((P, NSLOT, 1024), dtype=np.float32)
    r = np.arange(P)[:, None]
    j = np.arange(SPAN)[None, :]
    for i in range(NSLOT):
        qoff = SPAN * role_spans[i]
        for kq in range(4):
            key0 = P * (4 * i + kq)
            out[:, i, kq * SPAN : (kq + 1) * SPAN] = (qoff + j >= key0 + r)
    return out


def kernel(x, W_q, W_k, W_v, W_o):
    import ml_dtypes
    from concourse.bass_utils import run_bass_kernel_spmd

    bf16 = ml_dtypes.bfloat16
    nc = _get_program()

    x = np.asarray(x, dtype=np.float32)
    scale = np.float32(1.0 / np.sqrt(D_HEAD))
    wqkv = np.concatenate(
        [
            np.asarray(W_k, dtype=np.float32),
            np.asarray(W_v, dtype=np.float32),
            np.asarray(W_q, dtype=np.float32) * scale,
            np.zeros((D_MODEL, D_HEAD), np.float32),  # pad rows to 512B
        ],
        axis=1,
    ).astype(bf16)
    wo = np.asarray(W_o, dtype=np.float32).astype(bf16)

    masks = {
        0: _make_mask(ROLE_A).astype(bf16),
        1: _make_mask(ROLE_B).astype(bf16),
    }
    roles = {0: ROLE_A, 1: ROLE_B}

    in_maps = []
    for c in range(NCORES):
        b, role = c // 2, c % 2
        xtb = np.ascontiguousarray(x[b].T).astype(bf16)  # [1024, 4096]
        xtq = np.concatenate(
            [xtb[:, SPAN * j : SPAN * (j + 1)] for j in roles[role]], axis=1
        )
        in_maps.append(
            {
                "xt": xtb,
                "xtq": np.ascontiguousarray(xtq),
                "wqkv": wqkv,
                "wo": wo,
                "msk": masks[role],
            }
        )

    res = run_bass_kernel_spmd(nc, in_maps, core_ids=list(range(NCORES)))
    out = np.empty((BATCH, SEQ, D_MODEL), dtype=np.float32)
    for c in range(NCORES):
        b, role = c // 2, c % 2
        yc = res.results[c]["y"].astype(np.float32)  # [2048, 1024]
        for i, j in enumerate(roles[role]):
            out[b, SPAN * j : SPAN * (j + 1), :] = yc[
                SPAN * i : SPAN * (i + 1), :
            ]
    return out


# revision 71
# speedup vs baseline: 1.0555x; 1.0555x over previous
"""Trainium2 Bass kernel for single-head causal attention.

x:[4,4096,1024] f32, W_q/W_k/W_v:[1024,64], W_o:[64,1024].

Sharding: 8 cores = 4 batches x 2 roles. The 4096 queries of a batch are
split into 16 spans of 256; role A takes spans [0,3,4,7,8,11,12,15], role B
takes [1,2,5,6,9,10,13,14] (zig-zag), so both roles see an identical causal
workload profile: slot i (i=0..7, spans sorted by span index) attends to
exactly B_i = 4*(i+1) key chunks of 128. Per-core differences (which spans,
the causal-boundary masks) are carried purely in input data; the compiled
program is one SPMD binary.

All matmuls run in bf16 (1 cycle/row on the PE vs 4 for fp32) with fp32 PSUM
accumulation. Scores for 4 key chunks land in one [128,1024] PSUM quad and
get a single Exp activation; only the last quad of each slot needs a mask
multiply (covers both the causal diagonal and the dead blocks of the role
with the smaller span). Projection and output work for neighbouring slots is
interleaved between attention quads so the PE stays dense while the
activation engine paces the exp stream.
"""

import sys

for _p in ("/opt/trn_rl_repo",):
    if _p not in sys.path:
        sys.path.insert(0, _p)

import numpy as np

D_MODEL = 1024
D_HEAD = 64
SEQ = 4096
BATCH = 4
NCORES = 8
NQ = 2048           # queries per core
P = 128
DCH = D_MODEL // P  # 8 contraction chunks
SPAN = 256          # queries per slot
NSLOT = NQ // SPAN  # 8 slots per core
ROLE_A = [0, 3, 4, 7, 8, 11, 12, 15]
ROLE_B = [1, 2, 5, 6, 9, 10, 13, 14]

_prog = None


def _build_program():
    import concourse.bacc as bacc
    import concourse.mybir as mybir
    import concourse.tile as tile
    from concourse.masks import make_identity

    fp32 = mybir.dt.float32
    bf16 = mybir.dt.bfloat16
    nc = bacc.Bacc("TRN2", target_bir_lowering=False, debug=False)

    xt = nc.dram_tensor("xt", [D_MODEL, SEQ], bf16, kind="ExternalInput")
    xtq = nc.dram_tensor("xtq", [D_MODEL, NQ], bf16, kind="ExternalInput")
    wqkv = nc.dram_tensor("wqkv", [D_MODEL, 2 * P], bf16, kind="ExternalInput")
    wo = nc.dram_tensor("wo", [D_HEAD, D_MODEL], bf16, kind="ExternalInput")
    msk = nc.dram_tensor("msk", [P, NSLOT, 1024], bf16, kind="ExternalInput")
    y = nc.dram_tensor("y", [NQ, D_MODEL], bf16, kind="ExternalOutput")

    Exp = mybir.ActivationFunctionType.Exp

    with tile.TileContext(nc) as tc:
        with (
            tc.tile_pool(name="singles", bufs=1) as singles,
            tc.tile_pool(name="work", bufs=5) as work,
            tc.tile_pool(name="pwork", bufs=4) as pwork,
            tc.tile_pool(name="ywork", bufs=3) as ywork,
            tc.tile_pool(name="mm_ps", bufs=2, space="PSUM") as mm_ps,
            tc.tile_pool(name="s_ps", bufs=2, space="PSUM") as s_ps_pool,
            tc.tile_pool(name="pv_ps", bufs=2, space="PSUM") as pv_pool,
        ):
            # ---- persistent SBUF tensors ----
            kvt = singles.tile([P, SEQ], bf16, tag="kvt")  # 0:64 K^T, 64:128 V^T
            vaug = singles.tile([P, SEQ // P, D_HEAD + 1], bf16, tag="vaug")
            qt_sb = singles.tile([D_HEAD, NQ], bf16, tag="qt_sb")  # Q^T pre-scaled
            otb = singles.tile([D_HEAD, NQ], bf16, tag="otb")      # O^T (unscaled)
            rden = singles.tile([1, NQ], fp32, tag="rden")         # 1/den per query
            rdent = singles.tile([P, NQ // P], fp32, tag="rdent")  # transposed
            wpack_sb = singles.tile([P, DCH, 2 * P], bf16, tag="wpack_sb")
            wkv_sb = wpack_sb[:, :, 0:P]
            wq_sb = wpack_sb[:, :, P : P + D_HEAD]
            wo_sb = singles.tile([D_HEAD, D_MODEL], bf16, tag="wo_sb")
            msk_sb = singles.tile([P, NSLOT, 1024], bf16, tag="msk_sb")
            ident = singles.tile([P, D_HEAD], bf16, tag="ident")
            one_sb = singles.tile([1, 1], fp32, tag="one_sb")

            xc_t = [None] * 8   # x chunk tiles (keys)
            xq_t = [None] * 4   # x chunk tiles (queries)
            pv_t = [None] * 8   # per-slot PV accumulators
            kv_ps = {}
            q_ps = {}

            def dma_xc(c, first):
                """Load x^T key chunk c. The first chunk is split so the dc=0
                slab (all the first matmul needs) lands early."""
                src = xt[:, c * 512 : (c + 1) * 512].rearrange(
                    "(c p) m -> p c m", p=P
                )
                t = work.tile([P, DCH, 512], bf16, tag="xchunk", name=f"xc{c}")
                if first:
                    nc.sync.dma_start(out=t[:, 0:1, :], in_=src[:, 0:1, :])
                    nc.sync.dma_start(out=t[:, 1:4, :], in_=src[:, 1:4, :])
                    nc.sync.dma_start(out=t[:, 4:DCH, :], in_=src[:, 4:DCH, :])
                else:
                    nc.sync.dma_start(out=t[:, 0:4, :], in_=src[:, 0:4, :])
                    nc.sync.dma_start(out=t[:, 4:DCH, :], in_=src[:, 4:DCH, :])
                xc_t[c] = t

            def dma_xq(c, split=False):
                src_ = xtq[:, c * 512 : (c + 1) * 512].rearrange(
                    "(c p) m -> p c m", p=P
                )
                t = work.tile([P, DCH, 512], bf16, tag="xchunk", name=f"xq{c}")
                # column halves: each slot's queries land separately, so
                # the first half's projection never waits for the second
                nc.sync.dma_start(out=t[:, :, 0:256], in_=src_[:, :, 0:256])
                nc.sync.dma_start(out=t[:, :, 256:512], in_=src_[:, :, 256:512])
                xq_t[c] = t

            def dma_msk(s):
                nc.sync.dma_start(
                    out=msk_sb[:, s : s + 1, :], in_=msk[:, s : s + 1, :]
                )

            def proj_units(c):
                """K/V (+Q) projection for x chunk c, as small closures."""
                units = []

                def kv_mm(dc):
                    def go():
                        if dc == 0:
                            kv_ps[c] = mm_ps.tile([P, 512], fp32, tag="mm", name=f"kvps{c}")
                        nc.tensor.matmul(
                            kv_ps[c],
                            lhsT=wkv_sb[:, dc, :],
                            rhs=xc_t[c][:, dc, :],
                            start=(dc == 0),
                            stop=(dc == DCH - 1),
                            skip_group_check=True,
                        )
                    return go

                units += [kv_mm(dc) for dc in range(DCH)]
                units.append(
                    lambda: nc.vector.tensor_copy(
                        out=kvt[:, c * 512 : (c + 1) * 512], in_=kv_ps[c]
                    )
                )

                def transp():
                    tp = mm_ps.tile([P, 4, D_HEAD], bf16, tag="mm", name=f"tp{c}")
                    for t in range(4):
                        kc = c * 4 + t
                        nc.tensor.transpose(
                            tp[:, t, :],
                            kvt[D_HEAD:P, kc * P : (kc + 1) * P],
                            ident[D_HEAD:P, :],
                        )
                    nc.vector.tensor_copy(
                        out=vaug[:, c * 4 : c * 4 + 4, 0:D_HEAD], in_=tp
                    )

                units.append(transp)

                return units

            def qproj_units(c):
                """Q^T projection for query chunk c (slots 2c, 2c+1)."""
                units = []
                halves = ((0, 256), (256, 512))

                def q_mm(dc, h0, h1):
                    def go():
                        if dc == 0 and h0 == 0:
                            q_ps[c] = mm_ps.tile([D_HEAD, 512], fp32, tag="mm", name=f"qps{c}")
                        nc.tensor.matmul(
                            q_ps[c][:, h0:h1],
                            lhsT=wq_sb[:, dc, :],
                            rhs=xq_t[c][:, dc, h0:h1],
                            start=(dc == 0),
                            stop=(dc == DCH - 1),
                            skip_group_check=True,
                        )
                    return go

                def q_cp(h0, h1):
                    def go():
                        nc.vector.tensor_copy(
                            out=qt_sb[:, c * 512 + h0 : c * 512 + h1],
                            in_=q_ps[c][:, h0:h1],
                        )
                    return go

                for h0, h1 in halves:
                    units += [q_mm(dc, h0, h1) for dc in range(DCH)]
                    units.append(q_cp(h0, h1))
                return units

            def pre_units(s):
                """Per-slot epilogue part 1: free the PV accumulator —
                reciprocal of the denominator row, its transpose, O^T cast."""
                pv = pv_t[s]
                pr0 = s * SPAN
                units = [
                    lambda: nc.vector.reciprocal(
                        rden[:, pr0 : pr0 + SPAN], pv[D_HEAD : D_HEAD + 1, :]
                    ),
                ]

                def rdt():
                    rt_ps = mm_ps.tile([P, 2], fp32, tag="mm", name=f"rt{s}")
                    for k in range(2):
                        qti = s * 2 + k
                        nc.tensor.matmul(
                            rt_ps[:, k : k + 1],
                            lhsT=rden[:, qti * P : (qti + 1) * P],
                            rhs=one_sb,
                            start=True,
                            stop=True,
                        )
                    nc.vector.tensor_copy(
                        out=rdent[:, s * 2 : s * 2 + 2], in_=rt_ps
                    )

                units.append(rdt)
                units.append(
                    lambda: nc.vector.tensor_copy(
                        out=otb[:, pr0 : pr0 + SPAN], in_=pv[0:D_HEAD, :]
                    )
                )
                return units

            def out_units(s):
                """Per-slot epilogue part 2 (schedulable anywhere after
                pre_units(s)): out-projection, 1/den scaling, store. The
                final slot is latency-critical (nothing left to overlap
                with), so it takes wider PSUM tiles from the by-then-idle
                score pool, splits the scaling across DVE and ACT, and
                stores each half-row as soon as it is scaled."""
                last = s == 7
                yb_t = {}
                yp_t = {}

                def oproj(k, no):
                    def go():
                        qti = s * 2 + k
                        if no == 0:
                            yb_t[k] = ywork.tile([P, D_MODEL], bf16, tag="ysb", name=f"yb{s}_{k}")
                            if last:
                                yp_t[k] = s_ps_pool.tile(
                                    [P, 1024], fp32, tag="s", name=f"ypl{k}"
                                )
                        if last:
                            yp = yp_t[k][:, no * 512 : (no + 1) * 512]
                        else:
                            yp = mm_ps.tile([P, 512], fp32, tag="mm", name=f"yp{s}_{k}_{no}")
                        nc.tensor.matmul(
                            yp,
                            lhsT=otb[:, qti * P : (qti + 1) * P],
                            rhs=wo_sb[:, no * 512 : (no + 1) * 512],
                            start=True,
                            stop=True,
                        )
                        dst = yb_t[k][:, no * 512 : (no + 1) * 512]
                        if last and no == 1:
                            nc.scalar.activation(
                                dst,
                                yp,
                                mybir.ActivationFunctionType.Copy,
                                scale=rdent[:, qti : qti + 1],
                            )
                        else:
                            nc.vector.tensor_scalar_mul(
                                dst, yp, rdent[:, qti : qti + 1]
                            )
                        if last:
                            nc.sync.dma_start(
                                out=y[
                                    qti * P : (qti + 1) * P,
                                    no * 512 : (no + 1) * 512,
                                ],
                                in_=dst,
                            )
                        elif no == 1:
                            nc.sync.dma_start(
                                out=y[qti * P : (qti + 1) * P, :], in_=yb_t[k]
                            )
                    return go

                return [oproj(0, 0), oproj(0, 1), oproj(1, 0), oproj(1, 1)]

            # ---- prologue ----
            make_identity(nc, ident[D_HEAD:P, :])
            nc.vector.memset(one_sb, 1.0)
            nc.vector.memset(vaug[:, :, D_HEAD : D_HEAD + 1], 1.0)
            w_src = wqkv.rearrange("(c p) m -> p c m", p=P)
            nc.sync.dma_start(out=wpack_sb, in_=w_src)
            dma_xc(0, first=True)
            dma_xq(0, split=True)
            for u in proj_units(0) + qproj_units(0):
                u()

            # ---- main loop: attention(slot sc) + interleaved fillers ----
            # iteration sc hosts: projection of chunk sc+1, slot sc-1's PV
            # epilogue, and slot sc-2's output (deferred so the late,
            # exp-bound iterations get PE filler work); chunk DMAs are
            # issued two iterations ahead of their projection.
            for sc in range(8):
                if sc == 0:
                    dma_xc(1, first=False)
                    dma_msk(0)
                    dma_xc(2, first=False)
                    dma_xq(1)
                    dma_xq(2)
                    dma_msk(1)
                elif sc == 1:
                    dma_xc(3, first=False)
                    dma_xq(3)
                    nc.sync.dma_start(out=wo_sb, in_=wo[:, :])
                elif sc < 6:
                    dma_xc(sc + 2, first=False)

                units = []
                if sc < 7:
                    units += proj_units(sc + 1)
                    if sc + 1 < 4:
                        units += qproj_units(sc + 1)
                if sc >= 1:
                    units += pre_units(sc - 1)
                if sc >= 2:
                    units += out_units(sc - 2)
                tail_units = out_units(6) if sc == 7 else []

                B = 4 * (sc + 1)
                pv_t[sc] = pv_pool.tile(
                    [D_HEAD + 1, SPAN], fp32, tag="pv", name=f"pv{sc}"
                )
                pvh = pv_t[sc]
                qs_ap = qt_sb[:, sc * SPAN : (sc + 1) * SPAN]

                n_qd = sc + 1
                sq_t = [None] * n_qd
                p_t = [None] * n_qd

                def issue_qk(qd):
                    sq = s_ps_pool.tile([P, 1024], fp32, tag="s", name=f"sq{sc}_{qd}")
                    for t in range(4):
                        kc = 4 * qd + t
                        nc.tensor.matmul(
                            sq[:, t * SPAN : (t + 1) * SPAN],
                            lhsT=kvt[0:D_HEAD, kc * P : (kc + 1) * P],
                            rhs=qs_ap,
                            start=True,
                            stop=True,
                        )
                    sq_t[qd] = sq

                def issue_exp(qd):
                    p = pwork.tile([P, 1024], bf16, tag="p", name=f"p{sc}_{qd}")
                    nc.scalar.activation(p, sq_t[qd], Exp)
                    if qd == sc:  # last quad: causal boundary + dead blocks
                        nc.vector.tensor_tensor(
                            p, p, msk_sb[:, sc % 2, :], mybir.AluOpType.mult
                        )
                    p_t[qd] = p

                # masked quad second-to-last: off the slot-end critical
                # chain, but late enough that chunk sc's projection (a filler
                # in the previous iteration) has certainly landed.
                order = list(range(sc)) + [sc]
                if sc >= 1:
                    order[sc - 1], order[sc] = order[sc], order[sc - 1]

                def issue_pv(pos):
                    qd = order[pos]
                    for t in range(4):
                        kc = 4 * qd + t
                        nc.tensor.matmul(
                            pvh,
                            lhsT=vaug[:, kc, :],
                            rhs=p_t[qd][:, t * SPAN : (t + 1) * SPAN],
                            start=(pos == 0 and t == 0),
                            stop=(pos == n_qd - 1 and t == 3),
                            skip_group_check=True,
                        )

                # spread filler units across the quad stream
                ui = 0

                def fill(frac):
                    nonlocal ui
                    tgt = int(round(frac * len(units)))
                    while ui < tgt:
                        units[ui]()
                        ui += 1

                issue_qk(order[0])
                issue_exp(order[0])
                for pos in range(1, n_qd):
                    fill(pos / n_qd)
                    issue_qk(order[pos])
                    issue_pv(pos - 1)
                    issue_exp(order[pos])
                issue_pv(n_qd - 1)
                fill(1.0)
                for u in tail_units:
                    u()

            for u in pre_units(7) + out_units(7):
                u()

    nc.finalize()
    return nc


def _get_program():
    global _prog
    if _prog is None:
        _prog = _build_program()
    return _prog


def _make_mask(role_spans):
    """[128, 8, 1024] f32: slot i's last-quad window (key chunks 4i..4i+3)."""
    out = np.zeros((P, NSLOT, 1024), dtype=np.float32)
    r = np.arange(P)[:, None]
    j = np.arange(SPAN)[None, :]
    for i in range(NSLOT):
        qoff = SPAN * role_spans[i]
        for kq in range(4):
            key0 = P * (4 * i + kq)
            out[:, i, kq * SPAN : (kq + 1) * SPAN] = (qoff + j >= key0 + r)
    return out


def kernel(x, W_q, W_k, W_v, W_o):
    import ml_dtypes
    from concourse.bass_utils import run_bass_kernel_spmd

    bf16 = ml_dtypes.bfloat16
    nc = _get_program()

    x = np.asarray(x, dtype=np.float32)
    scale = np.float32(1.0 / np.sqrt(D_HEAD))
    wqkv = np.concatenate(
        [
            np.asarray(W_k, dtype=np.float32),
            np.asarray(W_v, dtype=np.float32),
            np.asarray(W_q, dtype=np.float32) * scale,
            np.zeros((D_MODEL, D_HEAD), np.float32),  # pad rows to 512B
        ],
        axis=1,
    ).astype(bf16)
    wo = np.asarray(W_o, dtype=np.float32).astype(bf16)

    masks = {
        0: _make_mask(ROLE_A).astype(bf16),
        1: _make_mask(ROLE_B).astype(bf16),
    }
    roles = {0: ROLE_A, 1: ROLE_B}

    in_maps = []
    for c in range(NCORES):
        b, role = c // 2, c % 2
        xtb = np.ascontiguousarray(x[b].T).astype(bf16)  # [1024, 4096]
        xtq = np.concatenate(
            [xtb[:, SPAN * j : SPAN * (j + 1)] for j in roles[role]], axis=1
        )
        in_maps.append(
            {
                "xt": xtb,
                "xtq": np.ascontiguousarray(xtq),
                "wqkv": wqkv,
                "wo": wo,
                "msk": masks[role],
            }
        )

    res = run_bass_kernel_spmd(nc, in_maps, core_ids=list(range(NCORES)))
    out = np.empty((BATCH, SEQ, D_MODEL), dtype=np.float32)
    for c in range(NCORES):
        b, role = c // 2, c % 2
        yc = res.results[c]["y"].astype(np.float32)  # [2048, 1024]
        for i, j in enumerate(roles[role]):
            out[b, SPAN * j : SPAN * (j + 1), :] = yc[
                SPAN * i : SPAN * (i + 1), :
            ]
    return out
